# revision 33
# baseline (speedup 1.0000x reference)
"""Causal self-attention (dense transformer block) on 8 Trainium2 NeuronCores.

Sharding: tensor-parallel over heads x data-parallel over batch.
Core i handles batch i//4 and heads 4*(i%4) .. 4*(i%4)+3. Per core:

  1. q^T,k^T = W_qk^T @ x^T           -- PE bf16, [64d x T] per head
  2. V token-major direct: x_tile.T @ W_v -> [128 tok, 4 heads x 64]
     cast fp8 (+ ones column for the softmax denominator)
  3. attention per (head, i-window of 1024), j-tiles paired for fp8
     DoubleRow PV: S^T[j,i] = k^T.T @ q^T (q pre-scaled), exp on ACT
     straight to fp8 P (two j-tiles = the two DoubleRow slots), causal
     via fp8 mask multiply + slot memsets, o_unnorm^T = [V|1]^T @ P
     with a 256-deep contraction per matmul (row 64 = denominator)
  4. normalize: DVE reciprocal of the denom row + K=1 broadcast matmul
     + DVE multiply -> o_norm^T in bf16
  5. partial_out = o_norm^T.T @ w_proj  (host sums partials + b_proj)

Emission pipelines S(pair p+1) ahead of PV(p) so ACT exp latency hides
behind PE work, and drains QKV/V/proj "filler" matmuls between pairs so
the PE never idles while ACT streams exps. All DMAs issue from the
SP/Pool sequencers so the ACT program never blocks on a DMA wait.
"""
import os
import sys

for _p in ("/opt/trn_rl_repo", "/root/.axon_site/_ro/trn_rl_repo"):
    if os.path.isdir(_p) and _p not in sys.path:
        sys.path.insert(0, _p)

import numpy as np
import ml_dtypes
import concourse.bass as bass
import concourse.tile as tile
from concourse import mybir
from concourse.bass_utils import run_bass_kernel_spmd

F32 = mybir.dt.float32
F32R = mybir.dt.float32r
BF16 = mybir.dt.bfloat16
FP8 = mybir.dt.float8e4
AF = mybir.ActivationFunctionType
MUL = mybir.AluOpType.mult
DR = mybir.MatmulPerfMode.DoubleRow

NP_BF16 = ml_dtypes.bfloat16
NP_FP8 = ml_dtypes.float8_e4m3

B, T, C = 2, 2048, 1024
HEADS, D = 16, 64
NCORES = 8
HPC = 4  # heads per core
SCALE = 1.0 / np.sqrt(np.float32(D))
WIN = 1024  # attention i-window
NW = T // WIN

MAX_WAITS = 1


def split_multi_waits(nc, max_waits=MAX_WAITS):
    """This walrus build rejects instructions carrying more than one sync-wait
    command. Split extras onto preceding same-engine NoOps (engine programs
    execute in order, so the gating is equivalent)."""
    idx = 0
    for f in nc.m.functions:
        for bb in f.blocks:
            out = []
            changed = False
            for inst in bb.instructions:
                si = inst.sync_info
                waits = list(si.on_wait) if si and si.on_wait else []
                if len(waits) > max_waits:
                    changed = True
                    for w in waits[:-max_waits]:
                        nop = mybir.InstNoOp(name=f"I-waitsplit-{idx}", ins=[], outs=[])
                        idx += 1
                        nop.engine = inst.engine
                        nop.sync_info = mybir.SyncInfo(on_wait=[w], on_update=[])
                        out.append(nop)
                    inst.sync_info = mybir.SyncInfo(
                        on_wait=waits[-max_waits:], on_update=list(si.on_update or [])
                    )
                out.append(inst)
            if changed:
                bb.instructions = out
    return nc


def build_nc(repeat=1, split=True):
    nc = bass.Bass(trn_type="TRN2")

    xT_d = nc.dram_tensor("xT", [C, T], BF16, kind="ExternalInput")
    wqk_d = nc.dram_tensor("wqk", [4, 128, 8, 128], BF16, kind="ExternalInput")
    bqk_d = nc.dram_tensor("bqk", [128, 4], F32, kind="ExternalInput")
    wv_d = nc.dram_tensor("wv", [128, 8, 256], BF16, kind="ExternalInput")
    bv_d = nc.dram_tensor("bv", [128, 256], F32, kind="ExternalInput")
    wproj_d = nc.dram_tensor("wproj", [2, 128, C], BF16, kind="ExternalInput")
    maskq_d = nc.dram_tensor("maskq", [128, 128], BF16, kind="ExternalInput")
    onesr_d = nc.dram_tensor("onesr", [1, 64], F32R, kind="ExternalInput")
    onesc_d = nc.dram_tensor("onesc", [128, 4, 16, 1], BF16, kind="ExternalInput")
    out_d = nc.dram_tensor("out", [T, C], F32, kind="ExternalOutput")

    # DMA issue rings: keep ACT/DVE sequencers free of DMA waits.
    rings = [nc.sync, nc.gpsimd]
    ring_i = [0]

    def ring():
        ring_i[0] += 1
        return rings[ring_i[0] % len(rings)]

    with tile.TileContext(nc) as tc:
        with tc.tile_pool(name="perm", bufs=1) as perm, \
             tc.tile_pool(name="pt", bufs=2) as ptp, \
             tc.tile_pool(name="rr", bufs=2) as rrp, \
             tc.tile_pool(name="rbc", bufs=2) as rbcp, \
             tc.tile_pool(name="sg", bufs=3) as sgp, \
             tc.tile_pool(name="ot1", bufs=2) as otmpp, \
             tc.tile_pool(name="psum", bufs=1, space="PSUM") as psp:
            xt = [perm.tile([128, T], BF16, tag=f"xt{ct}", name=f"xt{ct}")
                  for ct in range(8)]
            qkT = [perm.tile([128, T], BF16, tag=f"qkT{i}", name=f"qkT{i}")
                   for i in range(4)]  # q-hp0, q-hp1, k-hp0, k-hp1
            wqk = [perm.tile([128, 8, 128], BF16, tag=f"wqk{i}", name=f"wqk{i}")
                   for i in range(4)]
            wv_sb = perm.tile([128, 8, 256], BF16, tag="wv", name="wv")
            bqk_sb = perm.tile([128, 4], F32, tag="bqk", name="bqk")
            bv_sb = perm.tile([128, 256], F32, tag="bv", name="bv")
            wproj_sb = [perm.tile([128, C], BF16, tag=f"wp{i}", name=f"wp{i}")
                        for i in range(2)]
            maskq_sb = perm.tile([128, 128], BF16, tag="maskq", name="maskq")
            onesr_sb = perm.tile([1, 64], F32R, tag="onesr", name="onesr")
            vext = perm.tile([128, 4, 16, 65], BF16, tag="vext", name="vext")
            onorm = [perm.tile([128, T], BF16, tag=f"onorm{i}", name=f"onorm{i}")
                     for i in range(2)]

            # ---------------- emission helpers ----------------
            fillers = []

            def fill(n):
                for _ in range(n):
                    if not fillers:
                        return
                    fillers.pop(0)()

            def drain_fillers():
                while fillers:
                    fillers.pop(0)()

            def dma_x_chunk(ch):
                for ct in range(8):
                    ring().dma_start(
                        out=xt[ct][:, ch * 512:(ch + 1) * 512],
                        in_=xT_d[ct * 128:(ct + 1) * 128, ch * 512:(ch + 1) * 512],
                    )

            def qkv_block_chunk(blk, ch):
                """All 8 contraction matmuls for (block, 512-col chunk), then
                the bias-add -> bf16 SBUF. Atomic (one PSUM tile)."""
                ps = psp.tile([128, 512], F32, tag="spare", bufs=2, name="ps_qkv")
                for ct in range(8):
                    nc.tensor.matmul(
                        ps[:],
                        wqk[blk][:, ct, :],
                        xt[ct][:, ch * 512:(ch + 1) * 512],
                        start=(ct == 0),
                        stop=(ct == 7),
                    )
                nc.vector.tensor_scalar_add(
                    out=qkT[blk][:, ch * 512:(ch + 1) * 512],
                    in0=ps[:],
                    scalar1=bqk_sb[:, blk:blk + 1],
                )

            def qkv_chunk(ch, blocks=(0, 2, 1, 3)):
                for blk in blocks:
                    qkv_block_chunk(blk, ch)

            def vdir(tj):
                """V for token tile tj, all 4 heads: x_tile.T @ Wv, bias-add,
                cast fp8 into vext slots."""
                ps = psp.tile([128, 512], F32, tag="spare", bufs=2, name="ps_v")
                for ct in range(8):
                    nc.tensor.matmul(
                        ps[:, 0:256],
                        xt[ct][:, tj * 128:(tj + 1) * 128],
                        wv_sb[:, ct, :],
                        start=(ct == 0),
                        stop=(ct == 7),
                    )
                nc.vector.tensor_tensor(
                    out=vext[:, :, tj, 0:64], in0=ps[:, 0:256], in1=bv_sb[:],
                    op=mybir.AluOpType.add,
                )

            def att(h, w):
                """Attention for head h over i-window [w*WIN, (w+1)*WIN)."""
                hp, sub = h // 2, h % 2
                r0 = 64 * sub
                qb, kb = qkT[hp], qkT[2 + hp]
                wlo = w * WIN
                ntj = 8 * (w + 1)
                ot = psp.tile([65, WIN], F32, tag="ot", bufs=1, name="ot")
                lastt = [min(ntj - 1, 8 * w + 4 * t + 3) for t in range(2)]
                prev = None

                def pv(tj):
                    lo = max(0, 128 * tj - wlo)
                    pt = pts[tj % 2]
                    for t in range(2):
                        a, b = 512 * t, 512 * (t + 1)
                        if 512 * t + 511 < lo:
                            continue
                        if a < lo:
                            nc.gpsimd.memset(pt[:, a:lo], 0)
                        nc.tensor.matmul(
                            ot[:, a:b],
                            vext[:, h, tj, :],
                            pt[:, a:b],
                            start=(tj == 0),
                            stop=(tj == lastt[t]),
                        )

                pts = {}
                for tj in range(ntj):
                    j0 = 128 * tj
                    lo = max(0, j0 - wlo)
                    diag = j0 >= wlo
                    st = psp.tile([128, WIN], F32, tag="st", bufs=2, name="st")
                    for t in range(2):
                        if 512 * t + 511 >= lo:
                            nc.tensor.matmul(
                                st[:, 512 * t:512 * (t + 1)],
                                kb[r0:r0 + 64, j0:j0 + 128],
                                qb[r0:r0 + 64, wlo + 512 * t:wlo + 512 * (t + 1)],
                                start=True,
                                stop=True,
                            )
                    pt = ptp.tile([128, WIN], BF16, tag="pt", name="pt")
                    pts[tj % 2] = pt
                    nc.scalar.activation(
                        out=pt[:, lo:WIN], in_=st[:, lo:WIN], func=AF.Exp
                    )
                    if diag:
                        nc.vector.tensor_tensor(
                            out=pt[:, lo:lo + 128], in0=pt[:, lo:lo + 128],
                            in1=maskq_sb[:], op=MUL,
                        )
                    if prev is not None:
                        pv(prev)
                    if tj % 2 == 0:
                        fill(1)
                    prev = tj
                pv(prev)

                # normalize: recip of denom row, broadcast via K=1 matmul, mult
                rr = rrp.tile([1, WIN], F32R, tag="rr", name="rr")
                with nc.allow_low_precision("f32r is f32-width"):
                    nc.vector.reciprocal(out=rr[:], in_=ot[64:65, :])
                for t in range(2):
                    bcp = psp.tile([128, 512], F32, tag="spare", bufs=2, name="bc")
                    bc = bcp[0:64, :]
                    nc.tensor.matmul(bc, onesr_sb[:], rr[:, 512 * t:512 * (t + 1)],
                                     start=True, stop=True)
                    otun = rbcp.tile([64, 512], F32, tag="otun", name="otun")
                    nc.scalar.activation(
                        out=otun[:], in_=ot[0:64, 512 * t:512 * (t + 1)],
                        func=AF.Copy,
                    )
                    goff = wlo + 512 * t
                    if sub == 0:
                        dst = onorm[hp][0:64, goff:goff + 512]
                    else:
                        ot_s = otmpp.tile([64, 512], BF16, tag="otmp", name="ot_s")
                        dst = ot_s[:]
                    nc.vector.tensor_tensor(
                        out=dst, in0=otun[:], in1=bc, op=MUL,
                    )
                    if sub == 1:
                        nc.gpsimd.dma_start(
                            out=onorm[hp][64:128, goff:goff + 512], in_=ot_s[:]
                        )

            def proj(tb):
                for cc in range(2):
                    ps = psp.tile([128, 512], F32, tag="spare", bufs=2, name="ps_j")
                    for hp2 in range(2):
                        nc.tensor.matmul(
                            ps[:],
                            onorm[hp2][:, tb * 128:(tb + 1) * 128],
                            wproj_sb[hp2][:, cc * 512:(cc + 1) * 512],
                            start=(hp2 == 0),
                            stop=(hp2 == 1),
                        )
                    sg = sgp.tile([128, 512], F32, tag="sg", name="sg")
                    if cc == 0:
                        nc.vector.tensor_copy(out=sg[:], in_=ps[:])
                    else:
                        nc.scalar.activation(out=sg[:], in_=ps[:], func=AF.Copy)
                    ring().dma_start(
                        out=out_d[tb * 128:(tb + 1) * 128, cc * 512:(cc + 1) * 512],
                        in_=sg[:],
                    )

            # ---------------- master emission ----------------
            for _rep in range(repeat):
                nc.sync.dma_start(out=bqk_sb[:], in_=bqk_d[:])
                for blk in range(4):
                    ring().dma_start(out=wqk[blk][:], in_=wqk_d[blk])
                dma_x_chunk(0)
                nc.sync.dma_start(out=wv_sb[:], in_=wv_d[:])
                nc.gpsimd.dma_start(out=bv_sb[:], in_=bv_d[:])
                nc.sync.dma_start(out=maskq_sb[:], in_=maskq_d[:])
                nc.gpsimd.dma_start(out=onesr_sb[:], in_=onesr_d[:])
                nc.sync.dma_start(out=vext[:, :, :, 64:65], in_=onesc_d[:])
                dma_x_chunk(1)
                for i in range(2):
                    ring().dma_start(out=wproj_sb[i][:], in_=wproj_d[i])

                qkv_chunk(0)
                for tj in range(4):
                    vdir(tj)
                qkv_chunk(1)
                for tj in range(4, 8):
                    vdir(tj)

                # w0 attention with qkv chunks 2,3 + v tiles 8..15 as filler
                dma_x_chunk(2)
                for blk in (0, 2, 1, 3):
                    fillers.append(lambda b=blk: qkv_block_chunk(b, 2))
                att(0, 0)
                dma_x_chunk(3)
                for blk in (0, 2, 1, 3):
                    fillers.append(lambda b=blk: qkv_block_chunk(b, 3))
                att(1, 0)
                for tj in range(8, 16):
                    fillers.append(lambda j=tj: vdir(j))
                att(2, 0)
                att(3, 0)
                drain_fillers()

                # w1 attention with first-half proj as filler
                for tb in range(0, 8, 2):
                    fillers.append(lambda b=tb: proj(b))
                    fillers.append(lambda b=tb + 1: proj(b))
                att(0, 1)
                att(1, 1)
                att(2, 1)
                att(3, 1)
                drain_fillers()
                for tb in range(8, 16):
                    proj(tb)
    return split_multi_waits(nc) if split else nc


def host_shards(x, w_attn, b_attn, w_proj):
    """Per-core input maps. Pure layout/scalar transforms of the inputs."""
    x = np.asarray(x, dtype=np.float32)
    w_attn = np.asarray(w_attn, dtype=np.float32)
    b_attn = np.asarray(b_attn, dtype=np.float32)
    w_proj = np.asarray(w_proj, dtype=np.float32)

    maskq = np.triu(np.ones((128, 128), np.float32)).astype(NP_BF16)
    onesr = np.ones((1, 64), np.float32)
    onesc = np.ones((128, 4, 16, 1), np.float32).astype(NP_BF16)

    in_maps = []
    for core in range(NCORES):
        b = core // 4
        heads = [4 * (core % 4) + k for k in range(HPC)]
        # blocks: q-hp0, q-hp1, k-hp0, k-hp1 (2 heads x 64 cols each)
        wqk = np.zeros((4, 128, 8, 128), np.float32)
        bqk = np.zeros((128, 4), np.float32)
        for j, (base, scl) in enumerate(((0, SCALE), (C, 1.0))):
            for hpp in range(2):
                blk = 2 * j + hpp
                for s, h in enumerate((heads[2 * hpp], heads[2 * hpp + 1])):
                    cols = w_attn[:, base + h * D:base + (h + 1) * D] * scl
                    wqk[blk, :, :, s * 64:(s + 1) * 64] = (
                        cols.reshape(8, 128, D).transpose(1, 0, 2)
                    )
                    bqk[s * 64:(s + 1) * 64, blk] = (
                        b_attn[base + h * D:base + (h + 1) * D] * scl
                    )
        wv = np.concatenate(
            [w_attn[:, 2 * C + h * D:2 * C + (h + 1) * D] for h in heads], axis=1
        )  # [1024, 256] head-major cols
        wv = wv.reshape(8, 128, 256).transpose(1, 0, 2)  # [128, 8, 256]
        bv = np.broadcast_to(
            np.concatenate(
                [b_attn[2 * C + h * D:2 * C + (h + 1) * D] for h in heads]
            ),
            (128, 256),
        ).copy()  # [128, 256]
        wproj_s = np.stack([
            np.concatenate(
                [w_proj[h * D:(h + 1) * D, :] for h in heads[2 * hpp:2 * hpp + 2]],
                axis=0,
            )
            for hpp in range(2)
        ])  # [2, 128, 1024]
        xT = np.ascontiguousarray(x[b].T)  # [1024, 2048]
        in_maps.append(
            {
                "xT": xT.astype(NP_BF16),
                "wqk": wqk.astype(NP_BF16),
                "bqk": bqk,
                "wv": wv.astype(NP_BF16),
                "bv": bv,
                "wproj": wproj_s.astype(NP_BF16),
                "maskq": maskq,
                "onesr": onesr,
                "onesc": onesc,
            }
        )
    return in_maps


_NC = None


def _get_nc():
    global _NC
    if _NC is None:
        _NC = build_nc()
    return _NC


def kernel(x, w_attn, b_attn, w_proj, b_proj):
    in_maps = host_shards(x, w_attn, b_attn, w_proj)
    nc = _get_nc()
    res = run_bass_kernel_spmd(nc, in_maps, core_ids=list(range(NCORES)))
    out = np.zeros((B, T, C), np.float64)
    for core in range(NCORES):
        out[core // 4] += res.results[core]["out"].astype(np.float64)
    out += np.asarray(b_proj, dtype=np.float64)[None, None, :]
    return out.astype(np.float32)


# revision 35
# speedup vs baseline: 1.0021x; 1.0021x over previous
"""Causal self-attention (dense transformer block) on 8 Trainium2 NeuronCores.

Sharding: tensor-parallel over heads x data-parallel over batch.
Core i handles batch i//4 and heads 4*(i%4) .. 4*(i%4)+3. Per core:

  1. q^T,k^T = W_qk^T @ x^T           -- PE bf16, [64d x T] per head
  2. V token-major direct: x_tile.T @ W_v -> [128 tok, 4 heads x 64]
     cast fp8 (+ ones column for the softmax denominator)
  3. attention per (head, i-window of 1024), j-tiles paired for fp8
     DoubleRow PV: S^T[j,i] = k^T.T @ q^T (q pre-scaled), exp on ACT
     straight to fp8 P (two j-tiles = the two DoubleRow slots), causal
     via fp8 mask multiply + slot memsets, o_unnorm^T = [V|1]^T @ P
     with a 256-deep contraction per matmul (row 64 = denominator)
  4. normalize: DVE reciprocal of the denom row + K=1 broadcast matmul
     + DVE multiply -> o_norm^T in bf16
  5. partial_out = o_norm^T.T @ w_proj  (host sums partials + b_proj)

Emission pipelines S(pair p+1) ahead of PV(p) so ACT exp latency hides
behind PE work, and drains QKV/V/proj "filler" matmuls between pairs so
the PE never idles while ACT streams exps. All DMAs issue from the
SP/Pool sequencers so the ACT program never blocks on a DMA wait.
"""
import os
import sys

for _p in ("/opt/trn_rl_repo", "/root/.axon_site/_ro/trn_rl_repo"):
    if os.path.isdir(_p) and _p not in sys.path:
        sys.path.insert(0, _p)

import numpy as np
import ml_dtypes
import concourse.bass as bass
import concourse.tile as tile
from concourse import mybir
from concourse.bass_utils import run_bass_kernel_spmd

F32 = mybir.dt.float32
F32R = mybir.dt.float32r
BF16 = mybir.dt.bfloat16
FP8 = mybir.dt.float8e4
AF = mybir.ActivationFunctionType
MUL = mybir.AluOpType.mult
DR = mybir.MatmulPerfMode.DoubleRow

NP_BF16 = ml_dtypes.bfloat16
NP_FP8 = ml_dtypes.float8_e4m3

B, T, C = 2, 2048, 1024
HEADS, D = 16, 64
NCORES = 8
HPC = 4  # heads per core
SCALE = 1.0 / np.sqrt(np.float32(D))
WIN = 1024  # attention i-window
NW = T // WIN

MAX_WAITS = 1


def split_multi_waits(nc, max_waits=MAX_WAITS):
    """This walrus build rejects instructions carrying more than one sync-wait
    command. Split extras onto preceding same-engine NoOps (engine programs
    execute in order, so the gating is equivalent)."""
    idx = 0
    for f in nc.m.functions:
        for bb in f.blocks:
            out = []
            changed = False
            for inst in bb.instructions:
                si = inst.sync_info
                waits = list(si.on_wait) if si and si.on_wait else []
                if len(waits) > max_waits:
                    changed = True
                    for w in waits[:-max_waits]:
                        nop = mybir.InstNoOp(name=f"I-waitsplit-{idx}", ins=[], outs=[])
                        idx += 1
                        nop.engine = inst.engine
                        nop.sync_info = mybir.SyncInfo(on_wait=[w], on_update=[])
                        out.append(nop)
                    inst.sync_info = mybir.SyncInfo(
                        on_wait=waits[-max_waits:], on_update=list(si.on_update or [])
                    )
                out.append(inst)
            if changed:
                bb.instructions = out
    return nc


def build_nc(repeat=1, split=True):
    nc = bass.Bass(trn_type="TRN2")

    xT_d = nc.dram_tensor("xT", [C, T], BF16, kind="ExternalInput")
    wqk_d = nc.dram_tensor("wqk", [4, 128, 8, 128], BF16, kind="ExternalInput")
    bqk_d = nc.dram_tensor("bqk", [128, 4], F32, kind="ExternalInput")
    wv_d = nc.dram_tensor("wv", [128, 8, 256], BF16, kind="ExternalInput")
    bv_d = nc.dram_tensor("bv", [128, 256], F32, kind="ExternalInput")
    wproj_d = nc.dram_tensor("wproj", [2, 128, C], BF16, kind="ExternalInput")
    maskq_d = nc.dram_tensor("maskq", [128, 128], BF16, kind="ExternalInput")
    onesr_d = nc.dram_tensor("onesr", [1, 64], F32R, kind="ExternalInput")
    onesc_d = nc.dram_tensor("onesc", [128, 4, 16, 1], BF16, kind="ExternalInput")
    out_d = nc.dram_tensor("out", [T, C], F32, kind="ExternalOutput")

    # DMA issue rings: keep ACT/DVE sequencers free of DMA waits.
    rings = [nc.sync, nc.gpsimd]
    ring_i = [0]

    def ring():
        ring_i[0] += 1
        return rings[ring_i[0] % len(rings)]

    with tile.TileContext(nc) as tc:
        with tc.tile_pool(name="perm", bufs=1) as perm, \
             tc.tile_pool(name="pt", bufs=2) as ptp, \
             tc.tile_pool(name="rr", bufs=2) as rrp, \
             tc.tile_pool(name="rbc", bufs=2) as rbcp, \
             tc.tile_pool(name="sg", bufs=3) as sgp, \
             tc.tile_pool(name="ot1", bufs=2) as otmpp, \
             tc.tile_pool(name="psum", bufs=1, space="PSUM") as psp:
            xt = [perm.tile([128, T], BF16, tag=f"xt{ct}", name=f"xt{ct}")
                  for ct in range(8)]
            qkT = [perm.tile([128, T], BF16, tag=f"qkT{i}", name=f"qkT{i}")
                   for i in range(4)]  # q-hp0, q-hp1, k-hp0, k-hp1
            wqk = [perm.tile([128, 8, 128], BF16, tag=f"wqk{i}", name=f"wqk{i}")
                   for i in range(4)]
            wv_sb = perm.tile([128, 8, 256], BF16, tag="wv", name="wv")
            bqk_sb = perm.tile([128, 4], F32, tag="bqk", name="bqk")
            bv_sb = perm.tile([128, 256], F32, tag="bv", name="bv")
            wproj_sb = [perm.tile([128, C], BF16, tag=f"wp{i}", name=f"wp{i}")
                        for i in range(2)]
            maskq_sb = perm.tile([128, 128], BF16, tag="maskq", name="maskq")
            onesr_sb = perm.tile([1, 64], F32R, tag="onesr", name="onesr")
            vext = perm.tile([128, 4, 16, 65], BF16, tag="vext", name="vext")
            onorm = [perm.tile([128, T], BF16, tag=f"onorm{i}", name=f"onorm{i}")
                     for i in range(2)]

            # ---------------- emission helpers ----------------
            fillers = []

            def fill(n):
                for _ in range(n):
                    if not fillers:
                        return
                    fillers.pop(0)()

            def drain_fillers():
                while fillers:
                    fillers.pop(0)()

            def dma_x_chunk(ch):
                for ct in range(8):
                    ring().dma_start(
                        out=xt[ct][:, ch * 512:(ch + 1) * 512],
                        in_=xT_d[ct * 128:(ct + 1) * 128, ch * 512:(ch + 1) * 512],
                    )

            def qkv_block_chunk(blk, ch):
                """All 8 contraction matmuls for (block, 512-col chunk), then
                the bias-add -> bf16 SBUF. Atomic (one PSUM tile)."""
                ps = psp.tile([128, 512], F32, tag="spare", bufs=2, name="ps_qkv")
                for ct in range(8):
                    nc.tensor.matmul(
                        ps[:],
                        wqk[blk][:, ct, :],
                        xt[ct][:, ch * 512:(ch + 1) * 512],
                        start=(ct == 0),
                        stop=(ct == 7),
                    )
                nc.vector.tensor_scalar_add(
                    out=qkT[blk][:, ch * 512:(ch + 1) * 512],
                    in0=ps[:],
                    scalar1=bqk_sb[:, blk:blk + 1],
                )

            def qkv_chunk(ch, blocks=(0, 2, 1, 3)):
                for blk in blocks:
                    qkv_block_chunk(blk, ch)

            def vdir(tj):
                """V for token tile tj, all 4 heads: x_tile.T @ Wv, bias-add,
                cast fp8 into vext slots."""
                ps = psp.tile([128, 512], F32, tag="spare", bufs=2, name="ps_v")
                for ct in range(8):
                    nc.tensor.matmul(
                        ps[:, 0:256],
                        xt[ct][:, tj * 128:(tj + 1) * 128],
                        wv_sb[:, ct, :],
                        start=(ct == 0),
                        stop=(ct == 7),
                    )
                nc.vector.tensor_tensor(
                    out=vext[:, :, tj, 0:64], in0=ps[:, 0:256], in1=bv_sb[:],
                    op=mybir.AluOpType.add,
                )

            def att(h, w):
                """Attention for head h over i-window [w*WIN, (w+1)*WIN)."""
                hp, sub = h // 2, h % 2
                r0 = 64 * sub
                qb, kb = qkT[hp], qkT[2 + hp]
                wlo = w * WIN
                ntj = 8 * (w + 1)
                ot = psp.tile([65, WIN], F32, tag="ot", bufs=1, name="ot")
                lastt = [min(ntj - 1, 8 * w + 4 * t + 3) for t in range(2)]
                prev = None

                def pv(tj):
                    lo = max(0, 128 * tj - wlo)
                    pt = pts[tj % 2]
                    for t in range(2):
                        a, b = 512 * t, 512 * (t + 1)
                        if 512 * t + 511 < lo:
                            continue
                        if a < lo:
                            nc.gpsimd.memset(pt[:, a:lo], 0)
                        nc.tensor.matmul(
                            ot[:, a:b],
                            vext[:, h, tj, :],
                            pt[:, a:b],
                            start=(tj == 0),
                            stop=(tj == lastt[t]),
                        )

                pts = {}
                for tj in range(ntj):
                    j0 = 128 * tj
                    lo = max(0, j0 - wlo)
                    diag = j0 >= wlo
                    st = psp.tile([128, WIN], F32, tag="st", bufs=2, name="st")
                    for t in range(2):
                        if 512 * t + 511 >= lo:
                            nc.tensor.matmul(
                                st[:, 512 * t:512 * (t + 1)],
                                kb[r0:r0 + 64, j0:j0 + 128],
                                qb[r0:r0 + 64, wlo + 512 * t:wlo + 512 * (t + 1)],
                                start=True,
                                stop=True,
                            )
                    pt = ptp.tile([128, WIN], BF16, tag="pt", name="pt")
                    pts[tj % 2] = pt
                    nc.scalar.activation(
                        out=pt[:, lo:WIN], in_=st[:, lo:WIN], func=AF.Exp
                    )
                    if diag:
                        nc.vector.tensor_tensor(
                            out=pt[:, lo:lo + 128], in0=pt[:, lo:lo + 128],
                            in1=maskq_sb[:], op=MUL,
                        )
                    if prev is not None:
                        pv(prev)
                    if tj % 2 == 0:
                        fill(1)
                    prev = tj
                pv(prev)

                # normalize: recip of denom row, broadcast via K=1 matmul, mult
                rr = rrp.tile([1, WIN], F32R, tag="rr", name="rr")
                with nc.allow_low_precision("f32r is f32-width"):
                    nc.vector.reciprocal(out=rr[:], in_=ot[64:65, :])
                for t in range(2):
                    bcp = psp.tile([128, 512], F32, tag="spare", bufs=2, name="bc")
                    bc = bcp[0:64, :]
                    nc.tensor.matmul(bc, onesr_sb[:], rr[:, 512 * t:512 * (t + 1)],
                                     start=True, stop=True)
                    otun = rbcp.tile([64, 512], F32, tag="otun", name="otun")
                    nc.scalar.activation(
                        out=otun[:], in_=ot[0:64, 512 * t:512 * (t + 1)],
                        func=AF.Copy,
                    )
                    goff = wlo + 512 * t
                    if sub == 0:
                        dst = onorm[hp][0:64, goff:goff + 512]
                    else:
                        ot_s = otmpp.tile([64, 512], BF16, tag="otmp", name="ot_s")
                        dst = ot_s[:]
                    nc.vector.tensor_tensor(
                        out=dst, in0=otun[:], in1=bc, op=MUL,
                    )
                    if sub == 1:
                        nc.gpsimd.dma_start(
                            out=onorm[hp][64:128, goff:goff + 512], in_=ot_s[:]
                        )

            def proj(tb):
                for cc in range(2):
                    ps = psp.tile([128, 512], F32, tag="spare", bufs=2, name="ps_j")
                    for hp2 in range(2):
                        nc.tensor.matmul(
                            ps[:],
                            onorm[hp2][:, tb * 128:(tb + 1) * 128],
                            wproj_sb[hp2][:, cc * 512:(cc + 1) * 512],
                            start=(hp2 == 0),
                            stop=(hp2 == 1),
                        )
                    sg = sgp.tile([128, 512], F32, tag="sg", name="sg")
                    if cc == 0:
                        nc.vector.tensor_copy(out=sg[:], in_=ps[:])
                    else:
                        nc.scalar.activation(out=sg[:], in_=ps[:], func=AF.Copy)
                    ring().dma_start(
                        out=out_d[tb * 128:(tb + 1) * 128, cc * 512:(cc + 1) * 512],
                        in_=sg[:],
                    )

            # ---------------- master emission ----------------
            for _rep in range(repeat):
                nc.sync.dma_start(out=bqk_sb[:], in_=bqk_d[:])
                for blk in range(4):
                    ring().dma_start(out=wqk[blk][:], in_=wqk_d[blk])
                dma_x_chunk(0)
                nc.sync.dma_start(out=wv_sb[:], in_=wv_d[:])
                nc.gpsimd.dma_start(out=bv_sb[:], in_=bv_d[:])
                nc.sync.dma_start(out=maskq_sb[:], in_=maskq_d[:])
                nc.gpsimd.dma_start(out=onesr_sb[:], in_=onesr_d[:])
                nc.sync.dma_start(out=vext[:, :, :, 64:65], in_=onesc_d[:])
                dma_x_chunk(1)
                for i in range(2):
                    ring().dma_start(out=wproj_sb[i][:], in_=wproj_d[i])

                qkv_chunk(0)
                for tj in range(4):
                    vdir(tj)
                qkv_chunk(1)
                for tj in range(4, 8):
                    vdir(tj)

                # w0 attention with qkv chunks 2,3 + v tiles 8..15 as filler
                dma_x_chunk(2)
                for blk in (0, 2, 1, 3):
                    fillers.append(lambda b=blk: qkv_block_chunk(b, 2))
                att(0, 0)
                dma_x_chunk(3)
                for blk in (0, 2, 1, 3):
                    fillers.append(lambda b=blk: qkv_block_chunk(b, 3))
                att(1, 0)
                for tj in range(8, 16):
                    fillers.append(lambda j=tj: vdir(j))
                att(2, 0)
                att(3, 0)
                drain_fillers()

                # w1 attention: spread first-half proj across all four
                # ACT-bound w1 attentions so the PE filler never runs dry.
                for tb in (0, 1):
                    fillers.append(lambda b=tb: proj(b))
                att(0, 1)
                for tb in (2, 3):
                    fillers.append(lambda b=tb: proj(b))
                att(1, 1)
                for tb in (4, 5):
                    fillers.append(lambda b=tb: proj(b))
                att(2, 1)
                for tb in (6, 7):
                    fillers.append(lambda b=tb: proj(b))
                att(3, 1)
                drain_fillers()
                for tb in range(8, 16):
                    proj(tb)
    return split_multi_waits(nc) if split else nc


def host_shards(x, w_attn, b_attn, w_proj):
    """Per-core input maps. Pure layout/scalar transforms of the inputs."""
    x = np.asarray(x, dtype=np.float32)
    w_attn = np.asarray(w_attn, dtype=np.float32)
    b_attn = np.asarray(b_attn, dtype=np.float32)
    w_proj = np.asarray(w_proj, dtype=np.float32)

    maskq = np.triu(np.ones((128, 128), np.float32)).astype(NP_BF16)
    onesr = np.ones((1, 64), np.float32)
    onesc = np.ones((128, 4, 16, 1), np.float32).astype(NP_BF16)

    in_maps = []
    for core in range(NCORES):
        b = core // 4
        heads = [4 * (core % 4) + k for k in range(HPC)]
        # blocks: q-hp0, q-hp1, k-hp0, k-hp1 (2 heads x 64 cols each)
        wqk = np.zeros((4, 128, 8, 128), np.float32)
        bqk = np.zeros((128, 4), np.float32)
        for j, (base, scl) in enumerate(((0, SCALE), (C, 1.0))):
            for hpp in range(2):
                blk = 2 * j + hpp
                for s, h in enumerate((heads[2 * hpp], heads[2 * hpp + 1])):
                    cols = w_attn[:, base + h * D:base + (h + 1) * D] * scl
                    wqk[blk, :, :, s * 64:(s + 1) * 64] = (
                        cols.reshape(8, 128, D).transpose(1, 0, 2)
                    )
                    bqk[s * 64:(s + 1) * 64, blk] = (
                        b_attn[base + h * D:base + (h + 1) * D] * scl
                    )
        wv = np.concatenate(
            [w_attn[:, 2 * C + h * D:2 * C + (h + 1) * D] for h in heads], axis=1
        )  # [1024, 256] head-major cols
        wv = wv.reshape(8, 128, 256).transpose(1, 0, 2)  # [128, 8, 256]
        bv = np.broadcast_to(
            np.concatenate(
                [b_attn[2 * C + h * D:2 * C + (h + 1) * D] for h in heads]
            ),
            (128, 256),
        ).copy()  # [128, 256]
        wproj_s = np.stack([
            np.concatenate(
                [w_proj[h * D:(h + 1) * D, :] for h in heads[2 * hpp:2 * hpp + 2]],
                axis=0,
            )
            for hpp in range(2)
        ])  # [2, 128, 1024]
        xT = np.ascontiguousarray(x[b].T)  # [1024, 2048]
        in_maps.append(
            {
                "xT": xT.astype(NP_BF16),
                "wqk": wqk.astype(NP_BF16),
                "bqk": bqk,
                "wv": wv.astype(NP_BF16),
                "bv": bv,
                "wproj": wproj_s.astype(NP_BF16),
                "maskq": maskq,
                "onesr": onesr,
                "onesc": onesc,
            }
        )
    return in_maps


_NC = None


def _get_nc():
    global _NC
    if _NC is None:
        _NC = build_nc()
    return _NC


def kernel(x, w_attn, b_attn, w_proj, b_proj):
    in_maps = host_shards(x, w_attn, b_attn, w_proj)
    nc = _get_nc()
    res = run_bass_kernel_spmd(nc, in_maps, core_ids=list(range(NCORES)))
    out = np.zeros((B, T, C), np.float64)
    for core in range(NCORES):
        out[core // 4] += res.results[core]["out"].astype(np.float64)
    out += np.asarray(b_proj, dtype=np.float64)[None, None, :]
    return out.astype(np.float32)


# revision 36
# speedup vs baseline: 1.0280x; 1.0258x over previous
"""Causal self-attention (dense transformer block) on 8 Trainium2 NeuronCores.

Sharding: tensor-parallel over heads x data-parallel over batch.
Core i handles batch i//4 and heads 4*(i%4) .. 4*(i%4)+3. Per core:

  1. q^T,k^T = W_qk^T @ x^T           -- PE bf16, [64d x T] per head
  2. V token-major direct: x_tile.T @ W_v -> [128 tok, 4 heads x 64]
     cast fp8 (+ ones column for the softmax denominator)
  3. attention per (head, i-window of 1024), j-tiles paired for fp8
     DoubleRow PV: S^T[j,i] = k^T.T @ q^T (q pre-scaled), exp on ACT
     straight to fp8 P (two j-tiles = the two DoubleRow slots), causal
     via fp8 mask multiply + slot memsets, o_unnorm^T = [V|1]^T @ P
     with a 256-deep contraction per matmul (row 64 = denominator)
  4. normalize: DVE reciprocal of the denom row + K=1 broadcast matmul
     + DVE multiply -> o_norm^T in bf16
  5. partial_out = o_norm^T.T @ w_proj  (host sums partials + b_proj)

Emission pipelines S(pair p+1) ahead of PV(p) so ACT exp latency hides
behind PE work, and drains QKV/V/proj "filler" matmuls between pairs so
the PE never idles while ACT streams exps. All DMAs issue from the
SP/Pool sequencers so the ACT program never blocks on a DMA wait.
"""
import os
import sys

for _p in ("/opt/trn_rl_repo", "/root/.axon_site/_ro/trn_rl_repo"):
    if os.path.isdir(_p) and _p not in sys.path:
        sys.path.insert(0, _p)

import numpy as np
import ml_dtypes
import concourse.bass as bass
import concourse.tile as tile
from concourse import mybir
from concourse.bass_utils import run_bass_kernel_spmd

F32 = mybir.dt.float32
F32R = mybir.dt.float32r
BF16 = mybir.dt.bfloat16
FP8 = mybir.dt.float8e4
AF = mybir.ActivationFunctionType
MUL = mybir.AluOpType.mult
DR = mybir.MatmulPerfMode.DoubleRow

NP_BF16 = ml_dtypes.bfloat16
NP_FP8 = ml_dtypes.float8_e4m3

B, T, C = 2, 2048, 1024
HEADS, D = 16, 64
NCORES = 8
HPC = 4  # heads per core
SCALE = 1.0 / np.sqrt(np.float32(D))
WIN = 1024  # attention i-window
NW = T // WIN

MAX_WAITS = 1


def split_multi_waits(nc, max_waits=MAX_WAITS):
    """This walrus build rejects instructions carrying more than one sync-wait
    command. Split extras onto preceding same-engine NoOps (engine programs
    execute in order, so the gating is equivalent)."""
    idx = 0
    for f in nc.m.functions:
        for bb in f.blocks:
            out = []
            changed = False
            for inst in bb.instructions:
                si = inst.sync_info
                waits = list(si.on_wait) if si and si.on_wait else []
                if len(waits) > max_waits:
                    changed = True
                    for w in waits[:-max_waits]:
                        nop = mybir.InstNoOp(name=f"I-waitsplit-{idx}", ins=[], outs=[])
                        idx += 1
                        nop.engine = inst.engine
                        nop.sync_info = mybir.SyncInfo(on_wait=[w], on_update=[])
                        out.append(nop)
                    inst.sync_info = mybir.SyncInfo(
                        on_wait=waits[-max_waits:], on_update=list(si.on_update or [])
                    )
                out.append(inst)
            if changed:
                bb.instructions = out
    return nc


def build_nc(repeat=1, split=True):
    nc = bass.Bass(trn_type="TRN2")

    xT_d = nc.dram_tensor("xT", [C, T], BF16, kind="ExternalInput")
    wqk_d = nc.dram_tensor("wqk", [4, 128, 8, 128], BF16, kind="ExternalInput")
    bqk_d = nc.dram_tensor("bqk", [128, 4], F32, kind="ExternalInput")
    wv_d = nc.dram_tensor("wv", [128, 8, 256], BF16, kind="ExternalInput")
    bv_d = nc.dram_tensor("bv", [128, 256], F32, kind="ExternalInput")
    wproj_d = nc.dram_tensor("wproj", [2, 128, C], BF16, kind="ExternalInput")
    maskq_d = nc.dram_tensor("maskq", [128, 128], BF16, kind="ExternalInput")
    onesr_d = nc.dram_tensor("onesr", [1, 64], F32R, kind="ExternalInput")
    onesc_d = nc.dram_tensor("onesc", [128, 4, 16, 1], BF16, kind="ExternalInput")
    out_d = nc.dram_tensor("out", [T, C], F32, kind="ExternalOutput")

    # DMA issue rings: keep ACT/DVE sequencers free of DMA waits.
    rings = [nc.sync, nc.gpsimd]
    ring_i = [0]

    def ring():
        ring_i[0] += 1
        return rings[ring_i[0] % len(rings)]

    with tile.TileContext(nc) as tc:
        with tc.tile_pool(name="perm", bufs=1) as perm, \
             tc.tile_pool(name="pt", bufs=3) as ptp, \
             tc.tile_pool(name="rr", bufs=2) as rrp, \
             tc.tile_pool(name="rbc", bufs=2) as rbcp, \
             tc.tile_pool(name="sg", bufs=3) as sgp, \
             tc.tile_pool(name="ot1", bufs=2) as otmpp, \
             tc.tile_pool(name="psum", bufs=1, space="PSUM") as psp:
            xt = [perm.tile([128, T], BF16, tag=f"xt{ct}", name=f"xt{ct}")
                  for ct in range(8)]
            qkT = [perm.tile([128, T], BF16, tag=f"qkT{i}", name=f"qkT{i}")
                   for i in range(4)]  # q-hp0, q-hp1, k-hp0, k-hp1
            wqk = [perm.tile([128, 8, 128], BF16, tag=f"wqk{i}", name=f"wqk{i}")
                   for i in range(4)]
            wv_sb = perm.tile([128, 8, 256], BF16, tag="wv", name="wv")
            bqk_sb = perm.tile([128, 4], F32, tag="bqk", name="bqk")
            bv_sb = perm.tile([128, 256], F32, tag="bv", name="bv")
            wproj_sb = [perm.tile([128, C], BF16, tag=f"wp{i}", name=f"wp{i}")
                        for i in range(2)]
            maskq_sb = perm.tile([128, 128], BF16, tag="maskq", name="maskq")
            onesr_sb = perm.tile([1, 64], F32R, tag="onesr", name="onesr")
            vext = perm.tile([128, 4, 16, 65], BF16, tag="vext", name="vext")
            onorm = [perm.tile([128, T], BF16, tag=f"onorm{i}", name=f"onorm{i}")
                     for i in range(2)]

            # ---------------- emission helpers ----------------
            fillers = []

            def fill(n):
                for _ in range(n):
                    if not fillers:
                        return
                    fillers.pop(0)()

            def drain_fillers():
                while fillers:
                    fillers.pop(0)()

            def dma_x_chunk(ch):
                for ct in range(8):
                    ring().dma_start(
                        out=xt[ct][:, ch * 512:(ch + 1) * 512],
                        in_=xT_d[ct * 128:(ct + 1) * 128, ch * 512:(ch + 1) * 512],
                    )

            def qkv_block_chunk(blk, ch):
                """All 8 contraction matmuls for (block, 512-col chunk), then
                the bias-add -> bf16 SBUF. Atomic (one PSUM tile)."""
                ps = psp.tile([128, 512], F32, tag="spare", bufs=2, name="ps_qkv")
                for ct in range(8):
                    nc.tensor.matmul(
                        ps[:],
                        wqk[blk][:, ct, :],
                        xt[ct][:, ch * 512:(ch + 1) * 512],
                        start=(ct == 0),
                        stop=(ct == 7),
                    )
                nc.vector.tensor_scalar_add(
                    out=qkT[blk][:, ch * 512:(ch + 1) * 512],
                    in0=ps[:],
                    scalar1=bqk_sb[:, blk:blk + 1],
                )

            def qkv_chunk(ch, blocks=(0, 2, 1, 3)):
                for blk in blocks:
                    qkv_block_chunk(blk, ch)

            def vdir(tj):
                """V for token tile tj, all 4 heads: x_tile.T @ Wv, bias-add,
                cast fp8 into vext slots."""
                ps = psp.tile([128, 512], F32, tag="spare", bufs=2, name="ps_v")
                for ct in range(8):
                    nc.tensor.matmul(
                        ps[:, 0:256],
                        xt[ct][:, tj * 128:(tj + 1) * 128],
                        wv_sb[:, ct, :],
                        start=(ct == 0),
                        stop=(ct == 7),
                    )
                nc.vector.tensor_tensor(
                    out=vext[:, :, tj, 0:64], in0=ps[:, 0:256], in1=bv_sb[:],
                    op=mybir.AluOpType.add,
                )

            def att(h, w):
                """Attention for head h over i-window [w*WIN, (w+1)*WIN)."""
                hp, sub = h // 2, h % 2
                r0 = 64 * sub
                qb, kb = qkT[hp], qkT[2 + hp]
                wlo = w * WIN
                ntj = 8 * (w + 1)
                ot = psp.tile([65, WIN], F32, tag="ot", bufs=1, name="ot")
                lastt = [min(ntj - 1, 8 * w + 4 * t + 3) for t in range(2)]
                prev = None

                def pv(tj):
                    lo = max(0, 128 * tj - wlo)
                    pt = pts[tj % 2]
                    for t in range(2):
                        a, b = 512 * t, 512 * (t + 1)
                        if 512 * t + 511 < lo:
                            continue
                        if a < lo:
                            nc.gpsimd.memset(pt[:, a:lo], 0)
                        nc.tensor.matmul(
                            ot[:, a:b],
                            vext[:, h, tj, :],
                            pt[:, a:b],
                            start=(tj == 0),
                            stop=(tj == lastt[t]),
                        )

                pts = {}
                for tj in range(ntj):
                    j0 = 128 * tj
                    lo = max(0, j0 - wlo)
                    diag = j0 >= wlo
                    st = psp.tile([128, WIN], F32, tag="st", bufs=2, name="st")
                    for t in range(2):
                        if 512 * t + 511 >= lo:
                            nc.tensor.matmul(
                                st[:, 512 * t:512 * (t + 1)],
                                kb[r0:r0 + 64, j0:j0 + 128],
                                qb[r0:r0 + 64, wlo + 512 * t:wlo + 512 * (t + 1)],
                                start=True,
                                stop=True,
                            )
                    pt = ptp.tile([128, WIN], BF16, tag="pt", name="pt")
                    pts[tj % 2] = pt
                    nc.scalar.activation(
                        out=pt[:, lo:WIN], in_=st[:, lo:WIN], func=AF.Exp
                    )
                    if diag:
                        nc.vector.tensor_tensor(
                            out=pt[:, lo:lo + 128], in0=pt[:, lo:lo + 128],
                            in1=maskq_sb[:], op=MUL,
                        )
                    if prev is not None:
                        pv(prev)
                    if tj % 2 == 0:
                        fill(1)
                    prev = tj
                pv(prev)

                # normalize: recip of denom row, broadcast via K=1 matmul, mult
                rr = rrp.tile([1, WIN], F32R, tag="rr", name="rr")
                with nc.allow_low_precision("f32r is f32-width"):
                    nc.vector.reciprocal(out=rr[:], in_=ot[64:65, :])
                for t in range(2):
                    bcp = psp.tile([128, 512], F32, tag="spare", bufs=2, name="bc")
                    bc = bcp[0:64, :]
                    nc.tensor.matmul(bc, onesr_sb[:], rr[:, 512 * t:512 * (t + 1)],
                                     start=True, stop=True)
                    otun = rbcp.tile([64, 512], F32, tag="otun", name="otun")
                    nc.scalar.activation(
                        out=otun[:], in_=ot[0:64, 512 * t:512 * (t + 1)],
                        func=AF.Copy,
                    )
                    goff = wlo + 512 * t
                    if sub == 0:
                        dst = onorm[hp][0:64, goff:goff + 512]
                    else:
                        ot_s = otmpp.tile([64, 512], BF16, tag="otmp", name="ot_s")
                        dst = ot_s[:]
                    nc.vector.tensor_tensor(
                        out=dst, in0=otun[:], in1=bc, op=MUL,
                    )
                    if sub == 1:
                        nc.gpsimd.dma_start(
                            out=onorm[hp][64:128, goff:goff + 512], in_=ot_s[:]
                        )

            def proj(tb):
                for cc in range(2):
                    ps = psp.tile([128, 512], F32, tag="spare", bufs=2, name="ps_j")
                    for hp2 in range(2):
                        nc.tensor.matmul(
                            ps[:],
                            onorm[hp2][:, tb * 128:(tb + 1) * 128],
                            wproj_sb[hp2][:, cc * 512:(cc + 1) * 512],
                            start=(hp2 == 0),
                            stop=(hp2 == 1),
                        )
                    sg = sgp.tile([128, 512], F32, tag="sg", name="sg")
                    if cc == 0:
                        nc.vector.tensor_copy(out=sg[:], in_=ps[:])
                    else:
                        nc.scalar.activation(out=sg[:], in_=ps[:], func=AF.Copy)
                    ring().dma_start(
                        out=out_d[tb * 128:(tb + 1) * 128, cc * 512:(cc + 1) * 512],
                        in_=sg[:],
                    )

            # ---------------- master emission ----------------
            for _rep in range(repeat):
                nc.sync.dma_start(out=bqk_sb[:], in_=bqk_d[:])
                for blk in range(4):
                    ring().dma_start(out=wqk[blk][:], in_=wqk_d[blk])
                dma_x_chunk(0)
                nc.sync.dma_start(out=wv_sb[:], in_=wv_d[:])
                nc.gpsimd.dma_start(out=bv_sb[:], in_=bv_d[:])
                nc.sync.dma_start(out=maskq_sb[:], in_=maskq_d[:])
                nc.gpsimd.dma_start(out=onesr_sb[:], in_=onesr_d[:])
                nc.sync.dma_start(out=vext[:, :, :, 64:65], in_=onesc_d[:])
                dma_x_chunk(1)
                for i in range(2):
                    ring().dma_start(out=wproj_sb[i][:], in_=wproj_d[i])

                qkv_chunk(0)
                for tj in range(4):
                    vdir(tj)
                qkv_chunk(1)
                for tj in range(4, 8):
                    vdir(tj)

                # w0 attention with qkv chunks 2,3 + v tiles 8..15 as filler
                dma_x_chunk(2)
                for blk in (0, 2, 1, 3):
                    fillers.append(lambda b=blk: qkv_block_chunk(b, 2))
                att(0, 0)
                dma_x_chunk(3)
                for blk in (0, 2, 1, 3):
                    fillers.append(lambda b=blk: qkv_block_chunk(b, 3))
                att(1, 0)
                for tj in range(8, 16):
                    fillers.append(lambda j=tj: vdir(j))
                att(2, 0)
                att(3, 0)
                drain_fillers()

                # w1 attention: spread first-half proj across all four
                # ACT-bound w1 attentions so the PE filler never runs dry.
                for tb in (0, 1):
                    fillers.append(lambda b=tb: proj(b))
                att(0, 1)
                for tb in (2, 3):
                    fillers.append(lambda b=tb: proj(b))
                att(1, 1)
                for tb in (4, 5):
                    fillers.append(lambda b=tb: proj(b))
                att(2, 1)
                for tb in (6, 7):
                    fillers.append(lambda b=tb: proj(b))
                att(3, 1)
                drain_fillers()
                for tb in range(8, 16):
                    proj(tb)
    return split_multi_waits(nc) if split else nc


def host_shards(x, w_attn, b_attn, w_proj):
    """Per-core input maps. Pure layout/scalar transforms of the inputs."""
    x = np.asarray(x, dtype=np.float32)
    w_attn = np.asarray(w_attn, dtype=np.float32)
    b_attn = np.asarray(b_attn, dtype=np.float32)
    w_proj = np.asarray(w_proj, dtype=np.float32)

    maskq = np.triu(np.ones((128, 128), np.float32)).astype(NP_BF16)
    onesr = np.ones((1, 64), np.float32)
    onesc = np.ones((128, 4, 16, 1), np.float32).astype(NP_BF16)

    in_maps = []
    for core in range(NCORES):
        b = core // 4
        heads = [4 * (core % 4) + k for k in range(HPC)]
        # blocks: q-hp0, q-hp1, k-hp0, k-hp1 (2 heads x 64 cols each)
        wqk = np.zeros((4, 128, 8, 128), np.float32)
        bqk = np.zeros((128, 4), np.float32)
        for j, (base, scl) in enumerate(((0, SCALE), (C, 1.0))):
            for hpp in range(2):
                blk = 2 * j + hpp
                for s, h in enumerate((heads[2 * hpp], heads[2 * hpp + 1])):
                    cols = w_attn[:, base + h * D:base + (h + 1) * D] * scl
                    wqk[blk, :, :, s * 64:(s + 1) * 64] = (
                        cols.reshape(8, 128, D).transpose(1, 0, 2)
                    )
                    bqk[s * 64:(s + 1) * 64, blk] = (
                        b_attn[base + h * D:base + (h + 1) * D] * scl
                    )
        wv = np.concatenate(
            [w_attn[:, 2 * C + h * D:2 * C + (h + 1) * D] for h in heads], axis=1
        )  # [1024, 256] head-major cols
        wv = wv.reshape(8, 128, 256).transpose(1, 0, 2)  # [128, 8, 256]
        bv = np.broadcast_to(
            np.concatenate(
                [b_attn[2 * C + h * D:2 * C + (h + 1) * D] for h in heads]
            ),
            (128, 256),
        ).copy()  # [128, 256]
        wproj_s = np.stack([
            np.concatenate(
                [w_proj[h * D:(h + 1) * D, :] for h in heads[2 * hpp:2 * hpp + 2]],
                axis=0,
            )
            for hpp in range(2)
        ])  # [2, 128, 1024]
        xT = np.ascontiguousarray(x[b].T)  # [1024, 2048]
        in_maps.append(
            {
                "xT": xT.astype(NP_BF16),
                "wqk": wqk.astype(NP_BF16),
                "bqk": bqk,
                "wv": wv.astype(NP_BF16),
                "bv": bv,
                "wproj": wproj_s.astype(NP_BF16),
                "maskq": maskq,
                "onesr": onesr,
                "onesc": onesc,
            }
        )
    return in_maps


_NC = None


def _get_nc():
    global _NC
    if _NC is None:
        _NC = build_nc()
    return _NC


def kernel(x, w_attn, b_attn, w_proj, b_proj):
    in_maps = host_shards(x, w_attn, b_attn, w_proj)
    nc = _get_nc()
    res = run_bass_kernel_spmd(nc, in_maps, core_ids=list(range(NCORES)))
    out = np.zeros((B, T, C), np.float64)
    for core in range(NCORES):
        out[core // 4] += res.results[core]["out"].astype(np.float64)
    out += np.asarray(b_proj, dtype=np.float64)[None, None, :]
    return out.astype(np.float32)
